# revision 59
# baseline (speedup 1.0000x reference)
"""Causal self-attention with RoPE on 8 TRN2 NeuronCores — v3.

Sharding: core c -> (batch b = c//4, head-group g = c%4; 4 heads of 128 each).
Tensor-parallel over heads x data-parallel over batch.

v3 strategy vs v2:
  - QKV projections in fp8 (e4m3) DoubleRow matmuls with hi/lo error
    compensation: x and W are split on the host into fp8 hi + lo parts;
    W*x = Wh*xh + Wh*xl + Wl*xh, each DoubleRow packing a k-tile pair.
    Same accuracy as bf16 at a fraction of the PE time.
  - rowsum reoriented: p is the stationary matmul operand against a ones
    column, giving [128q, 1] outputs (tiny moving dim) accumulated in one
    psum bank; rinv is rebuilt into a row via a flattening DMA.
  - output projection also in fp8 DoubleRow: y is split on-chip into fp8
    hi/lo right after each head's normalize (y carries 16x scale via the
    rowsum-ones value so the fp8 range is well used).
  - next chunk's QKV and the previous chunk's projection are emitted
    interleaved between attention tiles, so the (in-order) PE queue has
    filler while exp chases on ACT; attention for the next chunk starts
    earlier and the collectives spread out.
  - psum evacuations alternate ACT/DVE (gpsimd cannot touch PSUM).
  - last chunk's projection is split: heads 0-1 partial-projected during
    the remaining attention (into an SBUF accumulator), re-injected via an
    identity matmul under the final head-2/3 pass.
"""
from contextlib import ExitStack

import numpy as np
import ml_dtypes

import concourse.bass as bass
import concourse.tile as tile
import concourse.mybir as mybir
from concourse import bacc, bass_utils

B = 2
S = 2048
D = 2048
NH, HD = 16, 128
HPC = 4                 # heads per core
EL = HPC * HD           # 512: local e-width per core
CH = 512                # token-chunk width
NCH = S // CH           # 4
DT = D // 128           # 16 d-tiles
ROPE_THETA = 10000.0
N_CORES = 8

F32 = mybir.dt.float32
F32R = mybir.dt.float32r
BF16 = mybir.dt.bfloat16
FP8 = mybir.dt.float8e4
AF = mybir.ActivationFunctionType
DR = mybir.MatmulPerfMode.DoubleRow

SX = 4.0      # fp8 scale for x
SW = 32.0     # fp8 scale for W_attn slices
QKV_SCALE = SX * SW          # q/k/v psums come out at 128x true scale
EXP_SCALE = float(HD ** -0.5)
SY = 16.0     # y carries 16x true scale (via the ones value 1/SY)
SP = 32.0     # fp8 scale for W_proj
Z_SCALE = SY * SP            # proj psums: 512x true scale


def _build():
    nc = bacc.Bacc("TRN2", target_bir_lowering=False, debug=False,
                   enable_asserts=True, num_devices=N_CORES)
    xh = nc.dram_tensor("xh", [D, S], FP8, kind="ExternalInput").ap()
    xl = nc.dram_tensor("xl", [D, S], FP8, kind="ExternalInput").ap()
    wqh = nc.dram_tensor("wqh", [D, EL], FP8, kind="ExternalInput").ap()
    wql = nc.dram_tensor("wql", [D, EL], FP8, kind="ExternalInput").ap()
    wkh = nc.dram_tensor("wkh", [D, EL], FP8, kind="ExternalInput").ap()
    wkl = nc.dram_tensor("wkl", [D, EL], FP8, kind="ExternalInput").ap()
    wvh = nc.dram_tensor("wvh", [D, EL], FP8, kind="ExternalInput").ap()
    wvl = nc.dram_tensor("wvl", [D, EL], FP8, kind="ExternalInput").ap()
    wph = nc.dram_tensor("wph", [EL, D], FP8, kind="ExternalInput").ap()
    wpl = nc.dram_tensor("wpl", [EL, D], FP8, kind="ExternalInput").ap()
    cosT = nc.dram_tensor("cosT", [HD, S], BF16, kind="ExternalInput").ap()
    sinT = nc.dram_tensor("sinT", [HD, S], BF16, kind="ExternalInput").ap()
    tri = nc.dram_tensor("tri", [128, 128], BF16, kind="ExternalInput").ap()
    ones = nc.dram_tensor("ones", [128, 1], BF16, kind="ExternalInput").ap()
    ident = nc.dram_tensor("ident", [128, 128], BF16, kind="ExternalInput").ap()
    zTc = nc.dram_tensor("zTc", [NCH * EL, CH], BF16,
                         kind="ExternalOutput").ap()

    with tile.TileContext(nc) as tc, \
         nc.allow_low_precision(reason="bf16 attention"), ExitStack() as ctx:
        # ---------------- pools ----------------
        cpool = ctx.enter_context(tc.tile_pool(name="const", bufs=1))
        wpool = ctx.enter_context(tc.tile_pool(name="w", bufs=1))
        xpool = ctx.enter_context(tc.tile_pool(name="x", bufs=2))
        kvres = ctx.enter_context(tc.tile_pool(name="kv", bufs=1))
        qpool = ctx.enter_context(tc.tile_pool(name="q", bufs=2))
        rope = ctx.enter_context(tc.tile_pool(name="rope", bufs=4))
        ppool = ctx.enter_context(tc.tile_pool(name="p", bufs=6))
        ypool = ctx.enter_context(tc.tile_pool(name="y", bufs=2))
        # yhi/ylo double-buffered: proj(ci) reads them while attn(ci+1)
        # already writes the next chunk's

        zpool = ctx.enter_context(tc.tile_pool(name="zacc", bufs=1))
        rpool = ctx.enter_context(tc.tile_pool(name="r", bufs=2))
        bpool = ctx.enter_context(tc.tile_pool(name="rbc", bufs=1))
        dram = ctx.enter_context(tc.tile_pool(name="dram", bufs=1, space="DRAM"))
        ps_mm = ctx.enter_context(tc.tile_pool(name="ps_mm", bufs=1, space="PSUM"))
        ps_q = ctx.enter_context(tc.tile_pool(name="ps_q", bufs=1, space="PSUM"))
        ps_s = ctx.enter_context(tc.tile_pool(name="ps_s", bufs=3, space="PSUM"))
        ps_o = ctx.enter_context(tc.tile_pool(name="ps_o", bufs=2, space="PSUM"))
        ps_r = ctx.enter_context(tc.tile_pool(name="ps_r", bufs=1, space="PSUM"))

        # ------------- weight / x loaders (split DMAs for pipelining) -------
        WSPLIT = 4            # d-tiles per weight sub-DMA

        def load_w(name, src, dt_, nsub):
            t = wpool.tile([128, DT * EL], FP8, name=name)
            step = DT // nsub
            for i in range(nsub):
                nc.sync.dma_start(
                    t[:, i * step * EL:(i + 1) * step * EL]
                        .rearrange("p (t e) -> p t e", t=step),
                    src.rearrange("(t p) e -> p t e", p=128)[:, i * step:(i + 1) * step, :])
            return t

        def load_wp(name, src, nt, wcols, nsub):
            t = wpool.tile([128, nt * wcols], FP8, name=name)
            step = nt // nsub
            for i in range(nsub):
                nc.sync.dma_start(
                    t[:, i * step * wcols:(i + 1) * step * wcols]
                        .rearrange("p (t e) -> p t e", t=step),
                    src.rearrange("(t p) e -> p t e", p=128)[:, i * step:(i + 1) * step, :])
            return t

        def load_x(ci):
            xht = xpool.tile([128, DT * CH], FP8, tag="xh", name=f"xh{ci}")
            xlt = xpool.tile([128, DT * CH], FP8, tag="xl", name=f"xl{ci}")
            nsub, step = 2, DT // 2
            for t, src in ((xht, xh), (xlt, xl)):
                for i in range(nsub):
                    nc.sync.dma_start(
                        t[:, i * step * CH:(i + 1) * step * CH]
                            .rearrange("p (t c) -> p t c", t=step),
                        src.rearrange("(t p) s -> p t s", p=128)
                          [:, i * step:(i + 1) * step, ci * CH:(ci + 1) * CH])
            return xht, xlt

        # startup order: wkh/xh first (the hi*hi matmuls need only those),
        # then wkl/xl, cos/sin (K rope), wq, wv, attention constants.
        wkh_sb = wpool.tile([128, DT * EL], FP8, name="wkh_sb")
        wkl_sb = wpool.tile([128, DT * EL], FP8, name="wkl_sb")
        xh_cur = xpool.tile([128, DT * CH], FP8, tag="xh", name="xh0")
        xl_cur = xpool.tile([128, DT * CH], FP8, tag="xl", name="xl0")
        for (i0, i1) in [(0, 2), (2, 6), (6, 16)]:
            nc.sync.dma_start(
                wkh_sb[:, i0 * EL:i1 * EL].rearrange("p (t e) -> p t e", t=i1 - i0),
                wkh.rearrange("(t p) e -> p t e", p=128)[:, i0:i1, :])
            nc.sync.dma_start(
                xh_cur[:, i0 * CH:i1 * CH].rearrange("p (t c) -> p t c", t=i1 - i0),
                xh.rearrange("(t p) s -> p t s", p=128)[:, i0:i1, 0:CH])
        for (i0, i1) in [(0, 8), (8, 16)]:
            nc.sync.dma_start(
                wkl_sb[:, i0 * EL:i1 * EL].rearrange("p (t e) -> p t e", t=i1 - i0),
                wkl.rearrange("(t p) e -> p t e", p=128)[:, i0:i1, :])
            nc.sync.dma_start(
                xl_cur[:, i0 * CH:i1 * CH].rearrange("p (t c) -> p t c", t=i1 - i0),
                xl.rearrange("(t p) s -> p t s", p=128)[:, i0:i1, 0:CH])
        cos_t = cpool.tile([HD, S], BF16)
        nc.sync.dma_start(cos_t[:], cosT)
        sin_t = cpool.tile([HD, S], BF16)
        nc.sync.dma_start(sin_t[:], sinT)
        wqh_sb = load_w("wqh", wqh, FP8, 2)
        wql_sb = load_w("wql", wql, FP8, 1)
        wvh_sb = load_w("wvh", wvh, FP8, 1)
        wvl_sb = load_w("wvl", wvl, FP8, 1)
        tri_t = cpool.tile([128, 128], BF16)
        nc.sync.dma_start(tri_t[:], tri)
        ones_t = cpool.tile([128, 1], BF16)
        nc.sync.dma_start(ones_t[:], ones)
        ident_t = cpool.tile([128, 128], BF16)
        nc.sync.dma_start(ident_t[:], ident)
        zero4_t = cpool.tile([128, 4], BF16)
        nc.vector.memset(zero4_t[:], 0)

        # ---------------- persistent K / V, z scratch ----------------
        k_c = [kvres.tile([HD, HPC * CH], BF16, name=f"k{ci}") for ci in range(NCH)]
        v_t = [kvres.tile([128, EL], BF16, name=f"v{st}") for st in range(S // 128)]
        z_part = {}
        z_rs = {}
        for ci in range(NCH):
            z_part[ci] = dram.tile([D, CH], BF16, tag=f"zp{ci}", name=f"zp{ci}")
            z_rs[ci] = dram.tile([EL, CH], BF16, tag=f"zr{ci}", name=f"zr{ci}")

        _QKV_POOLS = [(ps_s, "s_ps"), (ps_o, "o"), (ps_mm, "ps"), (ps_q, "q_ps")]
        _qkv_rot = [0]

        def qkv_psum_bulk(name):
            pool, tag = _QKV_POOLS[_qkv_rot[0] % 4]
            _qkv_rot[0] += 1
            return pool.tile([128, CH], F32, tag=tag, name=name)

        _il_rot = [0]

        def qkv_psum_il(name):
            pool, tag = (ps_q, "q_ps") if _il_rot[0] % 2 == 0 else (ps_mm, "ps")
            _il_rot[0] += 1
            return pool.tile([128, CH], F32, tag=tag, name=name)

        def rope_tail(ci, h, ps, out_ap, tagp):
            """psum [HD, CH] (at QKV_SCALE) -> RoPE -> out_ap (true scale via
            the 1/QKV_SCALE folded into the cos/sin tables)."""
            pre = rope.tile([HD, CH], BF16, tag="pre", name=f"pre_{tagp}")
            if h % 2 == 0:
                nc.scalar.copy(pre[:], ps[:])
            else:
                nc.vector.tensor_copy(pre[:], ps[:])
            rot = rope.tile([HD, CH], BF16, tag="rot", name=f"rot_{tagp}")
            nc.sync.dma_start(rot[0:64, :], pre[64:128, :])
            nc.sync.dma_start(rot[64:128, :], pre[0:64, :])
            cs = cos_t[:, ci * CH:(ci + 1) * CH]
            sn = sin_t[:, ci * CH:(ci + 1) * CH]
            t1 = rope.tile([HD, CH], BF16, tag="t1", name=f"t1_{tagp}")
            t2 = rope.tile([HD, CH], BF16, tag="t2", name=f"t2_{tagp}")
            nc.vector.tensor_mul(t1[:], pre[:], cs)
            nc.vector.tensor_mul(t2[:], rot[:], sn)
            nc.vector.tensor_add(out_ap, t1[:], t2[:])

        def gen_kq_unit(ci, h, x_hi, x_lo, w_hi, w_lo, out_ap, tagp, psup):
            """Generator: one head's fp8 hi/lo DR projection + rope.
            Yields every few matmuls so it can interleave with attention."""
            ps = psup(f"ps_{tagp}")
            wh = w_hi.rearrange("p (t e) -> p t e", t=DT)
            wl = w_lo.rearrange("p (t e) -> p t e", t=DT)
            xhr = x_hi.rearrange("p (t c) -> p t c", t=DT)
            xlr = x_lo.rearrange("p (t c) -> p t c", t=DT)
            hs = slice(h * HD, (h + 1) * HD)
            NP = DT // 2
            n = 0
            for term, (wt, xt) in enumerate(((wh, xhr), (wh, xlr), (wl, xhr))):
                for jj in range(NP):
                    ts = slice(2 * jj, 2 * jj + 2)
                    nc.tensor.matmul(
                        ps[:], wt[:, ts, hs], xt[:, ts, :],
                        start=(term == 0 and jj == 0),
                        stop=(term == 2 and jj == NP - 1), perf_mode=DR)
                    n += 1
                    if n % 3 == 0:
                        yield
            rope_tail(ci, h, ps, out_ap, tagp)
            yield

        def gen_v_unit(ci, st, x_hi, x_lo, psup):
            ps = psup(f"ps_v{ci}_{st}")
            wvh_r = wvh_sb.rearrange("p (t e) -> p t e", t=DT)
            wvl_r = wvl_sb.rearrange("p (t e) -> p t e", t=DT)
            xh_r = x_hi.rearrange("p (t c) -> p t c", t=DT)
            xl_r = x_lo.rearrange("p (t c) -> p t c", t=DT)
            ss = slice(st * 128, (st + 1) * 128)
            NP = DT // 2
            n = 0
            for term, (xt, wt) in enumerate(
                    ((xh_r, wvh_r), (xl_r, wvh_r), (xh_r, wvl_r))):
                for jj in range(NP):
                    ts = slice(2 * jj, 2 * jj + 2)
                    nc.tensor.matmul(
                        ps[:], xt[:, ts, ss], wt[:, ts, :],
                        start=(term == 0 and jj == 0),
                        stop=(term == 2 and jj == NP - 1), perf_mode=DR)
                    n += 1
                    if n % 3 == 0:
                        yield
            # v psum is at QKV_SCALE; rescale to true scale on evac
            if st % 2 == 0:
                nc.scalar.mul(v_t[ci * 4 + st][:], ps[:], 1.0 / QKV_SCALE)
            else:
                nc.vector.tensor_scalar_mul(v_t[ci * 4 + st][:], ps[:],
                                            1.0 / QKV_SCALE)
            yield

        def qkv_units(ci, x_hi, x_lo, psup, v_defer=0):
            q_sb = qpool.tile([128, HPC * CH], BF16, tag="q", name=f"q{ci}")
            gens = []
            for h in range(HPC):
                gens.append(gen_kq_unit(ci, h, x_hi, x_lo, wkh_sb, wkl_sb,
                                        k_c[ci][:, h * CH:(h + 1) * CH],
                                        f"k{ci}_{h}", psup))
            for h in range(HPC):
                gens.append(gen_kq_unit(ci, h, x_hi, x_lo, wqh_sb, wql_sb,
                                        q_sb[:, h * CH:(h + 1) * CH],
                                        f"q{ci}_{h}", psup))
            deferred = []
            for st in range(CH // 128):
                g = gen_v_unit(ci, st, x_hi, x_lo, psup)
                if st >= CH // 128 - v_defer:
                    deferred.append(g)
                else:
                    gens.append(g)
            return q_sb, gens, deferred

        class GQ:
            """A queue of emit-generators, pumped between attention tiles."""

            def __init__(self, gens=()):
                self.gens = list(gens)

            def pump(self, n):
                while n > 0 and self.gens:
                    try:
                        next(self.gens[0])
                        n -= 1
                    except StopIteration:
                        self.gens.pop(0)

            def drain(self):
                while self.gens:
                    try:
                        next(self.gens[0])
                    except StopIteration:
                        self.gens.pop(0)

        # ---------------- projection ----------------
        _PROJ_POOLS = [(ps_mm, "ps"), (ps_o, "o"), (ps_q, "q_ps"), (ps_o, "o")]

        def gen_proj(ci, y_hi, y_lo, cts, zacc, mode, pools):
            """Partial projection over head-tile pairs `cts` of chunk ci,
            in fp8 hi/lo DoubleRow form. mode: 'full' (evac+dma), 'acc0'
            (write zacc), 'fin' (identity-inject zacc, evac+dma)."""
            zp = z_part[ci]
            wh = wph_sb.rearrange("p (t e) -> p t e", t=HPC)
            wl = wpl_sb.rearrange("p (t e) -> p t e", t=HPC)
            yh = y_hi.rearrange("p (t c) -> p t c", t=HPC)
            yl = y_lo.rearrange("p (t c) -> p t c", t=HPC)
            pairs = [(a, a + 1) for a in cts[::2]]
            for eb in range(DT):
                pool, tag = pools[eb % len(pools)]
                ps = pool.tile([128, CH], F32, tag=tag, name=f"ps_z{ci}_{eb}_{mode}")
                if mode == "fin":
                    # re-inject the heads-0..1 partial via an identity matmul
                    # so the final z needs no vector-engine add chain
                    nc.tensor.matmul(ps[:], ident_t[:],
                                     zacc[:, eb * CH:(eb + 1) * CH],
                                     start=True, stop=False)
                es = slice(eb * 128, (eb + 1) * 128)
                n_mm = 3 * len(pairs)
                i = 0
                for (a, b) in pairs:
                    ts = slice(a, b + 1)
                    for (wt, yt) in ((wh, yh), (wh, yl), (wl, yh)):
                        nc.tensor.matmul(
                            ps[:], wt[:, ts, es], yt[:, ts, :],
                            start=(i == 0 and mode != "fin"),
                            stop=(i == n_mm - 1), perf_mode=DR)
                        i += 1
                if mode == "acc0":
                    nc.vector.tensor_copy(zacc[:, eb * CH:(eb + 1) * CH], ps[:])
                elif mode == "fin":
                    # rescale in place into the zacc staging; one grouped DMA
                    # per 4 ebs keeps the descriptor-generator off the tail
                    za = zacc[:, eb * CH:(eb + 1) * CH]
                    if eb % 2 == 0:
                        nc.scalar.mul(za, ps[:], 1.0 / Z_SCALE)
                    else:
                        nc.vector.tensor_scalar_mul(za, ps[:], 1.0 / Z_SCALE)
                    if eb % 4 == 3:
                        g = eb // 4
                        nc.sync.dma_start(
                            zp[g * 512:(g + 1) * 512, :]
                                .rearrange("(t p) c -> p t c", p=128),
                            zacc[:, g * 4 * CH:(g + 1) * 4 * CH]
                                .rearrange("p (t c) -> p t c", t=4))
                else:
                    zev = ppool.tile([128, CH], BF16, tag="zev",
                                     name=f"z{ci}_{eb}_{mode}")
                    if eb % 2 == 0:
                        nc.scalar.mul(zev[:], ps[:], 1.0 / Z_SCALE)
                    else:
                        nc.vector.tensor_scalar_mul(zev[:], ps[:], 1.0 / Z_SCALE)
                    nc.sync.dma_start(zp[eb * 128:(eb + 1) * 128, :], zev[:])
                yield

        def emit_rs(ci):
            # emitted with an unmerged (row, col) output AP: contiguous rows
            # stream through the collective engine row-by-row
            g = nc.gpsimd
            g.bass.has_collectives = True
            g.add_instruction(
                mybir.InstCollectiveCompute(
                    name=f"I-{g.bass.next_id()}",
                    kind="ReduceScatter",
                    op=mybir.AluOpType.add,
                    replica_groups=[[0, 1, 2, 3], [4, 5, 6, 7]],
                    ins=[g.lower_ap(z_part[ci].opt())],
                    outs=[g.lower_ap(z_rs[ci][:], opt=False)],
                    unique_tensors="No",
                    cc_dim="Partition"))

        # ---------------- attention ----------------
        def attn_chunk(ci, q_sb, filler, y_sb, y_hi, y_lo, on_head_done):
            """Causal attention for query chunk ci over key chunks 0..ci.
            `filler` (a GQ) is pumped between tiles to keep PE busy while
            exp chases on ACT."""
            n_jt = 4 * ci + 4
            tiles = [(h, jt) for h in range(HPC) for jt in range(n_jt)]
            state = {}
            pending = []

            def emit_or(ent):
                h, jt, p, off = ent
                o_ps, r_ps = state[h]
                nc.tensor.matmul(
                    o_ps[:, off:], v_t[jt][:, h * HD:(h + 1) * HD],
                    p[:, off:], start=(jt == 0), stop=(jt == n_jt - 1))
                # rowsum with p stationary: out [128q, 1] per query slice.
                # the bank is zeroed up front by a free 4-column matmul so
                # every accumulating column starts from a written region
                if jt == 0:
                    nc.tensor.matmul(r_ps[:, 0:4], tri_t[:], zero4_t[:],
                                     start=True, stop=False)
                for qs in range(off // 128, 4):
                    nc.tensor.matmul(
                        r_ps[:, qs:qs + 1], p[:, qs * 128:(qs + 1) * 128],
                        ones_t[:], start=False,
                        stop=(jt == n_jt - 1 and qs == 3))
                if jt == n_jt - 1:
                    # normalize head h: y = o * (1/rowsum); rinv arrives with
                    # queries on partitions, flattened to a row by DMA, then
                    # broadcast; the (q,j) interleave is undone by the read AP
                    rinv = rpool.tile([128, 4], BF16, tag="rinv")
                    nc.vector.reciprocal(rinv[:], r_ps[:, 0:4])
                    rrow = rpool.tile([1, CH], BF16, tag="rrow",
                                      name=f"rr{ci}_{h}")
                    nc.sync.dma_start(rrow[:], rinv[:])
                    rbc = bpool.tile([128, CH], BF16, tag="rbc", name=f"rb{ci}_{h}")
                    nc.gpsimd.partition_broadcast(rbc[:], rrow[:])
                    ysl = slice(h * CH, (h + 1) * CH)
                    nc.vector.tensor_mul(
                        y_sb[:, ysl], o_ps[:],
                        rbc.rearrange("p (q j) -> p j q", j=4))
                    # split y (16x true scale) into fp8 hi/lo for the fp8
                    # DoubleRow projection
                    if h % 2 == 0:
                        nc.scalar.copy(y_hi[:, ysl], y_sb[:, ysl])
                    else:
                        nc.vector.tensor_copy(y_hi[:, ysl], y_sb[:, ysl])
                    nc.vector.tensor_sub(y_lo[:, ysl], y_sb[:, ysl], y_hi[:, ysl])
                    del state[h]
                    on_head_done(h)

            n_t = len(tiles)
            for idx, (h, jt) in enumerate(tiles):
                if jt == 0:
                    o_ps = ps_o.tile([HD, CH], F32, tag="o", name=f"o{ci}_{h}")
                    r_ps = ps_r.tile([128, CH], F32, tag="r", name=f"r{ci}_{h}")
                    state[h] = (o_ps, r_ps)
                diag = jt - 4 * ci
                off = 128 * diag if diag > 0 else 0
                cj, j2 = divmod(jt, 4)
                s_ps = ps_s.tile([128, CH], F32, tag="s_ps", name=f"s{ci}_{h}_{jt}")
                nc.tensor.matmul(
                    s_ps[:, off:], k_c[cj][:, h * CH + j2 * 128:h * CH + (j2 + 1) * 128],
                    q_sb[:, h * CH + off:(h + 1) * CH], start=True, stop=True)
                p = ppool.tile([128, CH], BF16, tag="p")
                nc.scalar.activation(p[:, off:], s_ps[:, off:], AF.Exp,
                                     scale=EXP_SCALE)
                if 0 <= diag:
                    nc.vector.tensor_mul(
                        p[:, off:off + 128], p[:, off:off + 128], tri_t[:])
                if len(pending) >= 3:
                    emit_or(pending.pop(0))
                pending.append((h, jt, p, off))
                base = max(1, (len(filler.gens) * 7) // max(1, n_t - idx))
                filler.pump(base if ci < NCH - 1 else max(2, base))
            for ent in pending:
                emit_or(ent)

        def qkv_chunk0():
            """Chunk 0 with phase-split emission: all four heads' hi*hi
            matmuls first (they need only the hi tensors, which load first),
            then the lo cross terms + rope while the rest streams in."""
            q_sb = qpool.tile([128, HPC * CH], BF16, tag="q", name="q0")
            NP = DT // 2
            xhr = xh_cur.rearrange("p (t c) -> p t c", t=DT)
            xlr = xl_cur.rearrange("p (t c) -> p t c", t=DT)
            for (w_hi, w_lo, outf, tag) in (
                    (wkh_sb, wkl_sb, lambda h: k_c[0][:, h * CH:(h + 1) * CH], "k"),
                    (wqh_sb, wql_sb, lambda h: q_sb[:, h * CH:(h + 1) * CH], "q")):
                wh = w_hi.rearrange("p (t e) -> p t e", t=DT)
                wl = w_lo.rearrange("p (t e) -> p t e", t=DT)
                pss = {}
                for h in range(HPC):
                    pss[h] = qkv_psum_bulk(f"ps_{tag}0_{h}")
                    hs = slice(h * HD, (h + 1) * HD)
                    for jj in range(NP):
                        ts = slice(2 * jj, 2 * jj + 2)
                        nc.tensor.matmul(
                            pss[h][:], wh[:, ts, hs], xhr[:, ts, :],
                            start=(jj == 0), stop=False, perf_mode=DR)
                for h in range(HPC):
                    hs = slice(h * HD, (h + 1) * HD)
                    for term, (wt, xt) in enumerate(((wh, xlr), (wl, xhr))):
                        for jj in range(NP):
                            ts = slice(2 * jj, 2 * jj + 2)
                            nc.tensor.matmul(
                                pss[h][:], wt[:, ts, hs], xt[:, ts, :],
                                start=False,
                                stop=(term == 1 and jj == NP - 1),
                                perf_mode=DR)
                    rope_tail(0, h, pss[h], outf(h), f"{tag}0_{h}")
            for st in range(CH // 128):
                for _ in gen_v_unit(0, st, xh_cur, xl_cur, qkv_psum_bulk):
                    pass
            return q_sb

        # ---------------- main loop ----------------
        q_cur = qkv_chunk0()
        xh_cur, xl_cur = load_x(1)
        wph_sb = load_wp("wph", wph, EL // 128, D, 2)
        wpl_sb = load_wp("wpl", wpl, EL // 128, D, 2)

        carry = []
        prev_proj = None
        for ci in range(NCH):
            y_sb = ypool.tile([128, HPC * CH], BF16, tag="y", name=f"y{ci}")
            y_hi = ypool.tile([128, HPC * CH], FP8, tag="yhi", name=f"yh{ci}")
            y_lo = ypool.tile([128, HPC * CH], FP8, tag="ylo", name=f"yl{ci}")
            last = ci == NCH - 1
            filler = GQ()
            q_next = None
            if not last:
                defer = 2 if ci + 1 == NCH - 1 else 0
                q_next, gens, deferred = qkv_units(ci + 1, xh_cur, xl_cur,
                                                   qkv_psum_il, defer)
                filler.gens.extend(gens)
            else:
                filler.gens.extend(carry)
            if not last:
                carry = deferred
            if prev_proj is not None:
                filler.gens.append(prev_proj)
            zacc = None
            if last:
                zacc = zpool.tile([128, DT * CH], BF16, name="zacc")

            def on_head_done(h, ci=ci, y_hi=y_hi, y_lo=y_lo, zacc=zacc,
                             filler=filler, last=last):
                if last and h == 1:
                    filler.gens.append(
                        gen_proj(ci, y_hi, y_lo, (0, 1), zacc, "acc0",
                                 [(ps_mm, "ps"), (ps_q, "q_ps")]))

            attn_chunk(ci, q_cur, filler, y_sb, y_hi, y_lo, on_head_done)
            filler.drain()
            if prev_proj is not None:
                emit_rs(ci - 1)
                prev_proj = None
            if last:
                GQ([gen_proj(ci, y_hi, y_lo, (2, 3), zacc, "fin",
                             _PROJ_POOLS)]).drain()
                emit_rs(ci)
            else:
                prev_proj = gen_proj(ci, y_hi, y_lo, (0, 1, 2, 3), None,
                                     "full", [(ps_mm, "ps"), (ps_q, "q_ps")])
            if ci + 2 <= NCH - 1:
                xh_cur, xl_cur = load_x(ci + 2)
            q_cur = q_next

        # RS cannot target an ExternalOutput; DRAM->DRAM DMAs move the
        # scattered slices into the output tensor. Emitted at the very end so
        # bounces 0..2 fire immediately and only the last waits on RS(3).
        for ci in range(NCH):
            nc.sync.dma_start(zTc[ci * EL:(ci + 1) * EL, :], z_rs[ci][:])
    nc.compile()
    return nc


def _tables():
    inv_freq = 1.0 / (ROPE_THETA ** (np.arange(0, HD, 2, dtype=np.float64) / HD))
    pos = np.arange(S, dtype=np.float64)
    f_half = np.outer(inv_freq, pos)                  # [64, S]
    freqs = np.concatenate([f_half, f_half], axis=0)  # [HD, S]
    emb32 = freqs.astype(np.float32)
    cos_t = np.cos(emb32) / QKV_SCALE
    sin_t = np.sin(emb32) / QKV_SCALE
    sgn = np.where(np.arange(HD) < HD // 2, -1.0, 1.0).astype(np.float32)[:, None]
    return cos_t.astype(ml_dtypes.bfloat16), (sin_t * sgn).astype(ml_dtypes.bfloat16)


def _split8(t, scale):
    """Split scale*t into fp8 hi + lo with hi = fp8(scale*t)."""
    t = np.asarray(t, np.float32) * np.float32(scale)
    hi = t.astype(ml_dtypes.float8_e4m3)
    lo = (t - hi.astype(np.float32)).astype(ml_dtypes.float8_e4m3)
    return np.ascontiguousarray(hi), np.ascontiguousarray(lo)


_NC_CACHE = {}


def _get_nc():
    if "nc" not in _NC_CACHE:
        _NC_CACHE["nc"] = _build()
    return _NC_CACHE["nc"]


def make_in_maps(x, W_attn, W_proj):
    x = np.asarray(x, dtype=np.float32)
    W_attn = np.asarray(W_attn, dtype=np.float32)
    W_proj = np.asarray(W_proj, dtype=np.float32)
    cos_t, sin_t = _tables()
    tri = np.triu(np.ones((128, 128), np.float32)).astype(ml_dtypes.bfloat16)
    ones = np.full((128, 1), 1.0 / SY, ml_dtypes.bfloat16)
    xb = [_split8(x[b].T, SX) for b in range(B)]
    ident = np.eye(128, dtype=np.float32).astype(ml_dtypes.bfloat16)
    wps = {g: _split8(W_proj[g * EL:(g + 1) * EL, :], SP) for g in range(HPC)}
    wspl = {}
    for g in range(HPC):
        wspl[g] = {
            "wq": _split8(W_attn[:, g * EL:(g + 1) * EL], SW),
            "wk": _split8(W_attn[:, D + g * EL:D + (g + 1) * EL], SW),
            "wv": _split8(W_attn[:, 2 * D + g * EL:2 * D + (g + 1) * EL], SW),
        }
    in_maps = []
    for c in range(N_CORES):
        b, g = divmod(c, HPC)
        in_maps.append({
            "xh": xb[b][0], "xl": xb[b][1],
            "wqh": wspl[g]["wq"][0], "wql": wspl[g]["wq"][1],
            "wkh": wspl[g]["wk"][0], "wkl": wspl[g]["wk"][1],
            "wvh": wspl[g]["wv"][0], "wvl": wspl[g]["wv"][1],
            "wph": wps[g][0], "wpl": wps[g][1],
            "cosT": cos_t, "sinT": sin_t,
            "tri": tri, "ones": ones, "ident": ident,
        })
    return in_maps


def assemble(results):
    out = np.empty((B, S, D), dtype=np.float32)
    for c in range(N_CORES):
        b, g = divmod(c, HPC)
        z = np.asarray(results[c]["zTc"]).astype(np.float32)   # [NCH*EL, CH]
        for ci in range(NCH):
            out[b, ci * CH:(ci + 1) * CH, g * EL:(g + 1) * EL] = \
                z[ci * EL:(ci + 1) * EL, :].T
    return out


def kernel(x, W_attn, W_proj):
    nc = _get_nc()
    in_maps = make_in_maps(x, W_attn, W_proj)
    res = bass_utils.run_bass_kernel_spmd(
        nc, in_maps, core_ids=list(range(N_CORES)), trace=False)
    return assemble(res.results)


if __name__ == "__main__":
    rng = np.random.default_rng(0)
    x = rng.standard_normal((B, S, D)).astype(np.float32)
    W_attn = (rng.standard_normal((D, 3 * D)) * D ** -0.5).astype(np.float32)
    W_proj = (rng.standard_normal((D, D)) * D ** -0.5).astype(np.float32)
    out = kernel(x, W_attn, W_proj)
    print("out", out.shape, out.dtype, np.abs(out).mean())
